# revision 1
# baseline (speedup 1.0000x reference)
"""CutCrossEntropyLoss (sampled softmax, 512 noise + 1 target per token) on 8 trn2 cores.

Strategy (data-parallel over the 1024 flattened tokens, 128/core):
 - Host: cast classifier W to bf16 into an augmented table [zero; W; zero]
   (50259 rows).  Per token, the 513 sampled rows (1 target + 512 noise) are
   split into two fixed-size index lists addressed from two base offsets of
   the table so every index fits dma_gather's int16 limit:
       half A: table rows [0, 32766]      (vocab v <= 32766), 256 slots
       half B: table rows [17492, 50258]  (vocab v >= 17490), 288 slots
   Unused slots point at an all-zero row, so their logits are exactly 0 and
   are harmless in the loss reductions (exp(0 - max) ~ 0, sum += 0).  The
   target row sits at column 0 of whichever half can address it.
 - Device: dma_gather(transpose=True) lands gathered rows K-major
   ([128 hidden, 6 chunks, n_idx]) -- directly usable as matmul rhs.  Per
   token, 12 accumulating M=1 bf16 matmuls produce its 544 logits in a PSUM
   row; 4 tokens run concurrently in the PE's four 32-column groups (PSUM
   rows 0/32/64/96).  Each round's PSUM is drained full-width into column
   segment r of an SBUF stage tile [128, 32*544] (only rows {0,32,64,96}
   carry data; engines require 32-aligned partition bases, so the unused
   rows just compute garbage that the host ignores).  Free-dim segmented
   reductions + Exp give per-token max / sum(exp) / sum(logits) and the
   loss, laid out [128, 32].
 - Host: pick rows {0,32,64,96}, mean the 1024 per-token losses.
"""
import sys

sys.path.insert(0, "/opt/trn_rl_repo")

import numpy as np
import ml_dtypes

H = 768
KC = 6  # H / 128
V = 50257
NTOK = 1024
SAMPLE = 512
NCORES = 8
TPC = 128  # tokens per core

ACAP = 256
BCAP = 384  # 256 + 128: gather calls are capped at 256 idxs (proven HW size)
B1 = 256
B2 = 128
SLOTS = ACAP + BCAP  # 640
BASE1 = 17492  # row offset of gather-half B within the augmented table
VA = 50259  # augmented table rows: [zero, W(50257), zero]
ZB = 32766  # pad row for half B (absolute row 50258); half A pads to row 0

T_CH = 4  # tokens per gather chunk (one 4-token PE round per chunk)
NCH = TPC // T_CH  # 32 chunks == 32 rounds
LS = 0.1
NPROB = LS / SAMPLE

_CACHE = {}


def _wrap_idx(flat):
    """dma_gather index layout: idx i at [i % 16, i // 16], replicated to 128 partitions."""
    n = flat.shape[0]
    w = flat.reshape(n // 16, 16).T  # [16, n/16]
    return np.tile(w, (8, 1))  # [128, n/16]


def _build_bass():
    import concourse.bacc as bacc
    import concourse.mybir as mybir
    from concourse import tile

    nc = bacc.Bacc("TRN2", debug=False, num_devices=NCORES, num_swdge_queues=2)
    f32 = mybir.dt.float32
    bf16 = mybir.dt.bfloat16
    i16 = mybir.dt.int16
    AX = mybir.AxisListType.X
    OP = mybir.AluOpType
    ACTF = mybir.ActivationFunctionType

    w_aug = nc.dram_tensor("w_aug", [VA, H], bf16, kind="ExternalInput")
    idxa = nc.dram_tensor("idxa", [128, TPC * (ACAP // 16)], i16, kind="ExternalInput")
    idxb1 = nc.dram_tensor("idxb1", [128, TPC * (B1 // 16)], i16, kind="ExternalInput")
    idxb2 = nc.dram_tensor("idxb2", [128, TPC * (B2 // 16)], i16, kind="ExternalInput")
    ht = nc.dram_tensor("ht", [128, KC * 128], bf16, kind="ExternalInput")
    tmask = nc.dram_tensor("tmask", [128, NCH], f32, kind="ExternalInput")
    loss_out = nc.dram_tensor("loss", [128, NCH], f32, kind="ExternalOutput")

    with tile.TileContext(nc) as tc:
        with (
            tc.tile_pool(name="const", bufs=1) as cpool,
            tc.tile_pool(name="gath", bufs=3) as gpool,
            tc.tile_pool(name="ps", bufs=3, space="PSUM") as ppool,
            tc.tile_pool(name="work", bufs=1) as wpool,
        ):
            idxa_t = cpool.tile([128, TPC * (ACAP // 16)], i16)
            nc.sync.dma_start(out=idxa_t[:], in_=idxa[:])
            idxb1_t = cpool.tile([128, TPC * (B1 // 16)], i16)
            nc.sync.dma_start(out=idxb1_t[:], in_=idxb1[:])
            idxb2_t = cpool.tile([128, TPC * (B2 // 16)], i16)
            nc.sync.dma_start(out=idxb2_t[:], in_=idxb2[:])
            ht_t = cpool.tile([128, KC, 128], bf16)
            nc.sync.dma_start(out=ht_t[:], in_=ht[:].rearrange("p (c t) -> p c t", c=KC))
            tmask_t = cpool.tile([128, NCH], f32)
            nc.sync.dma_start(out=tmask_t[:], in_=tmask[:])

            stage = wpool.tile([128, NCH, SLOTS], f32)
            nc.vector.memset(stage[:], 0.0)

            for ch in range(NCH):
                ga = gpool.tile([128, T_CH, KC, ACAP], bf16, tag="ga")
                gb1 = gpool.tile([128, T_CH, KC, B1], bf16, tag="gb1")
                gb2 = gpool.tile([128, T_CH, KC, B2], bf16, tag="gb2")
                for j in range(T_CH):
                    tok = ch * T_CH + j
                    nc.gpsimd.dma_gather(
                        out_ap=ga[:, j, :, :],
                        in_ap=w_aug[:, :],
                        idxs_ap=idxa_t[:, tok * (ACAP // 16) : (tok + 1) * (ACAP // 16)],
                        num_idxs=ACAP,
                        num_idxs_reg=ACAP,
                        elem_size=H,
                        transpose=True,
                        queue_num=0,
                    )
                    nc.gpsimd.dma_gather(
                        out_ap=gb1[:, j, :, :],
                        in_ap=w_aug[BASE1:, :],
                        idxs_ap=idxb1_t[:, tok * (B1 // 16) : (tok + 1) * (B1 // 16)],
                        num_idxs=B1,
                        num_idxs_reg=B1,
                        elem_size=H,
                        transpose=True,
                        queue_num=1,
                    )
                    nc.gpsimd.dma_gather(
                        out_ap=gb2[:, j, :, :],
                        in_ap=w_aug[BASE1:, :],
                        idxs_ap=idxb2_t[:, tok * (B2 // 16) : (tok + 1) * (B2 // 16)],
                        num_idxs=B2,
                        num_idxs_reg=B2,
                        elem_size=H,
                        transpose=True,
                        queue_num=1,
                    )
                psa = ppool.tile([128, ACAP], f32, tag="pa")
                psb = ppool.tile([128, BCAP], f32, tag="pb")
                for j in range(4):
                    tok = ch * T_CH + j
                    for c in range(KC):
                        nc.tensor.matmul(
                            out=psa[32 * j : 32 * j + 1, :],
                            lhsT=ht_t[:, c, tok : tok + 1],
                            rhs=ga[:, j, c, :],
                            start=(c == 0),
                            stop=(c == KC - 1),
                            tile_position=(0, 32 * j),
                        )
                    for c in range(KC):
                        nc.tensor.matmul(
                            out=psb[32 * j : 32 * j + 1, 0:B1],
                            lhsT=ht_t[:, c, tok : tok + 1],
                            rhs=gb1[:, j, c, :],
                            start=(c == 0),
                            stop=(c == KC - 1),
                            tile_position=(0, 32 * j),
                        )
                    for c in range(KC):
                        nc.tensor.matmul(
                            out=psb[32 * j : 32 * j + 1, B1:BCAP],
                            lhsT=ht_t[:, c, tok : tok + 1],
                            rhs=gb2[:, j, c, :],
                            start=(c == 0),
                            stop=(c == KC - 1),
                            tile_position=(0, 32 * j),
                        )
                # drain the four written PSUM rows (32-aligned bases are required)
                for j in range(4):
                    nc.scalar.copy(
                        out=stage[32 * j : 32 * j + 1, ch, 0:ACAP],
                        in_=psa[32 * j : 32 * j + 1, :],
                    )
                    nc.vector.tensor_copy(
                        out=stage[32 * j : 32 * j + 1, ch, ACAP:SLOTS],
                        in_=psb[32 * j : 32 * j + 1, :],
                    )

            negmx = wpool.tile([128, NCH], f32)
            nc.vector.tensor_reduce(
                out=negmx[:], in_=stage[:], axis=AX, op=OP.max, negate=True
            )
            ssum = wpool.tile([128, NCH], f32)
            nc.vector.tensor_reduce(out=ssum[:], in_=stage[:], axis=AX, op=OP.add)
            lta = wpool.tile([128, NCH], f32)
            nc.vector.tensor_copy(out=lta[:], in_=stage[:, :, 0])
            ltb = wpool.tile([128, NCH], f32)
            nc.vector.tensor_copy(out=ltb[:], in_=stage[:, :, ACAP])

            # stage <- exp(stage - max)
            nc.vector.tensor_tensor(
                out=stage[:],
                in0=stage[:],
                in1=negmx[:].to_broadcast([128, NCH, SLOTS]),
                op=OP.add,
            )
            nc.scalar.activation(
                out=stage[:].rearrange("p a b -> p (a b)"),
                in_=stage[:].rearrange("p a b -> p (a b)"),
                func=ACTF.Exp,
            )
            sexp = wpool.tile([128, NCH], f32)
            nc.vector.tensor_reduce(out=sexp[:], in_=stage[:], axis=AX, op=OP.add)

            # lt = A0 + tmask * (B0 - A0)
            lt = wpool.tile([128, NCH], f32)
            nc.vector.tensor_tensor(out=lt[:], in0=ltb[:], in1=lta[:], op=OP.subtract)
            nc.vector.tensor_tensor(out=lt[:], in0=lt[:], in1=tmask_t[:], op=OP.mult)
            nc.vector.tensor_tensor(out=lt[:], in0=lt[:], in1=lta[:], op=OP.add)

            # lse = max + ln(sexp) = ln(sexp) - negmx
            lse = wpool.tile([128, NCH], f32)
            nc.scalar.activation(out=lse[:], in_=sexp[:], func=ACTF.Ln)
            nc.vector.tensor_tensor(out=lse[:], in0=lse[:], in1=negmx[:], op=OP.subtract)

            # loss = lse - 0.9*lt - NPROB*(ssum - lt)
            nsum = wpool.tile([128, NCH], f32)
            nc.vector.tensor_tensor(out=nsum[:], in0=ssum[:], in1=lt[:], op=OP.subtract)
            tmp = wpool.tile([128, NCH], f32)
            nc.vector.tensor_scalar_mul(out=tmp[:], in0=lt[:], scalar1=-(1.0 - LS))
            nc.vector.tensor_tensor(out=lse[:], in0=lse[:], in1=tmp[:], op=OP.add)
            nc.vector.tensor_scalar_mul(out=tmp[:], in0=nsum[:], scalar1=-NPROB)
            nc.vector.tensor_tensor(out=lse[:], in0=lse[:], in1=tmp[:], op=OP.add)

            nc.sync.dma_start(out=loss_out[:], in_=lse[:])

    nc.compile()
    return nc


def _prep_inputs(hidden_states, weight, target, noise_indx):
    h = np.asarray(hidden_states, np.float32).reshape(NTOK, H)
    W = np.asarray(weight, np.float32)
    tgt = np.asarray(target).reshape(NTOK).astype(np.int64)
    nz = np.asarray(noise_indx).astype(np.int64)

    w_aug = np.zeros((VA, H), dtype=ml_dtypes.bfloat16)
    w_aug[1 : V + 1] = W.astype(ml_dtypes.bfloat16)

    aug = nz + 1  # [NTOK, 512] augmented row ids
    tga = tgt + 1
    ta = tga <= 32766  # target addressable from half A

    lista = np.zeros((NTOK, ACAP), np.int16)
    listb = np.full((NTOK, BCAP), ZB, np.int16)  # BCAP=384
    for n in range(NTOK):
        a = aug[n]
        must_a = a < BASE1
        must_b = a > 32766
        flex = ~must_a & ~must_b
        fa = a[must_a]
        fb = a[must_b]
        fl = a[flex]
        cap_a = ACAP - 1 if ta[n] else ACAP
        take = min(cap_a - fa.shape[0], fl.shape[0])
        assert take >= 0 and fb.shape[0] + (fl.shape[0] - take) <= (
            BCAP - (0 if ta[n] else 1)
        ), f"token {n}: split infeasible"
        arow = np.concatenate([fa, fl[:take]])
        brow = np.concatenate([fb, fl[take:]])
        if ta[n]:
            lista[n, 0] = tga[n]
            lista[n, 1 : 1 + arow.shape[0]] = arow
            listb[n, : brow.shape[0]] = brow - BASE1
        else:
            listb[n, 0] = tga[n] - BASE1
            listb[n, 1 : 1 + brow.shape[0]] = brow - BASE1
            lista[n, : arow.shape[0]] = arow

    in_maps = []
    for core in range(NCORES):
        sl = slice(core * TPC, (core + 1) * TPC)
        la = lista[sl]  # [128, 256]
        lb = listb[sl]  # [128, 384]
        ia = np.hstack([_wrap_idx(la[t]) for t in range(TPC)])
        ib1 = np.hstack([_wrap_idx(lb[t, :B1]) for t in range(TPC)])
        ib2 = np.hstack([_wrap_idx(lb[t, B1:]) for t in range(TPC)])
        hc = h[sl].astype(ml_dtypes.bfloat16)  # [128, 768]
        htc = np.ascontiguousarray(
            hc.reshape(TPC, KC, 128).transpose(2, 1, 0)
        ).reshape(128, KC * 128)
        # tmask[32j, ch] = target-in-B for token ch*4+j of this core
        tm = np.zeros((128, NCH), np.float32)
        tb = (~ta[sl]).astype(np.float32).reshape(NCH, T_CH)  # [ch, j]
        for j in range(4):
            tm[32 * j, :] = tb[:, j]
        in_maps.append(
            {"w_aug": w_aug, "idxa": ia, "idxb1": ib1, "idxb2": ib2, "ht": htc,
             "tmask": tm}
        )
    return in_maps


def _unpack_losses(results):
    losses = []
    for c in range(NCORES):
        out = np.asarray(results[c]["loss"], np.float32)  # [128, 32]
        per_tok = out[[0, 32, 64, 96], :].T.reshape(-1)  # token ch*4+j at [j, ch]
        losses.append(per_tok)
    return np.concatenate(losses)


def kernel(hidden_states, weight, target, noise_indx):
    from concourse.bass_utils import run_bass_kernel_spmd

    if "nc" not in _CACHE:
        _CACHE["nc"] = _build_bass()
    nc = _CACHE["nc"]
    in_maps = _prep_inputs(hidden_states, weight, target, noise_indx)
    res = run_bass_kernel_spmd(nc, in_maps, core_ids=list(range(NCORES)))
    return np.float32(_unpack_losses(res.results).mean())



# revision 5
# speedup vs baseline: 11.5511x; 11.5511x over previous
"""CutCrossEntropyLoss (sampled softmax, 512 noise + 1 target per token) on 8 trn2 cores.

Strategy (vocab-parallel: shard the 50257-row classifier over the 8 cores,
6288 rows each; upload the weight exactly once, in fp8_e4m3):
 - Each core holds its W-shard transposed ([128 hdim, 6 chunks, 6288 vocab],
   fp8) and ALL 1024 tokens' hidden states transposed (fp8), both resident in
   SBUF.  It computes the full logits block h @ Wc^T for its shard: 8 token
   groups x 13 vocab tiles x 6 accumulating K=128 matmuls into PSUM, drained
   to an SBUF stage tile [128 tokens, 6288] f32.
 - The sampled-softmax reductions only need, per token: the target logit plus
   the 512 noise logits (with multiplicity).  Host packs, per (token, 1572-
   vocab subrange), the distinct sampled local indices with their counts c and
   linear weights w2 = NPROB*c + (0.9-NPROB)*[v==target]; the device expands
   them to dense rows C / W2 [128 tokens, 6288] bf16 with gpsimd.local_scatter
   (zero background, -1 pads ignored).
 - Per token group: rowmax m (unmasked shard max, a valid stabilizer), then
   L2 = sum(W2 * x) via fused tensor_tensor_reduce, exp in place via scalar
   activation with bias=-m, then Z = sum(C * exp(x-m)).  Each core returns
   [128, 8 groups x (negmax, Z, L2)] f32.
 - Host combine (tiny): per token M = max_c m_c, Ztot = sum_c Z_c*exp(m_c-M),
   loss = M + log(Ztot) - sum_c L2_c; mean over the 1024 tokens.
   (loss = lse - 0.9*lt - NPROB*(ssum - lt) and sum_c L2_c equals
   0.9*lt + NPROB*(ssum - lt) because target is always in the sampled set.)
"""
import sys

sys.path.insert(0, "/opt/trn_rl_repo")

import numpy as np
import ml_dtypes

H = 768
KC = 6  # H / 128
V = 50257
NTOK = 1024
SAMPLE = 512
NCORES = 8

SH = 6288  # vocab rows per core shard (8 * 6288 = 50304 >= 50257; pad rows are zero)
NSUB = 4  # local_scatter subranges per shard
SUBW = SH // NSUB  # 1572 (< 2048 gpsimd local_scatter limit)
NG = 8  # token groups of 128
TPG = 128
KSLOT = 48  # packed (idx, val) slots per (token, subrange); actual max ~33
NT = (SH + 511) // 512  # 13 vocab tiles (12 x 512 + 144)

LS = 0.1
NPROB = LS / SAMPLE

_CACHE = {}


def _build_bass():
    import concourse.bacc as bacc
    import concourse.mybir as mybir
    from concourse import tile

    nc = bacc.Bacc("TRN2", debug=False, num_devices=NCORES)
    f32 = mybir.dt.float32
    bf16 = mybir.dt.bfloat16
    fp8 = mybir.dt.float8e4
    i16 = mybir.dt.int16
    AX = mybir.AxisListType.X
    OP = mybir.AluOpType
    ACTF = mybir.ActivationFunctionType

    wt = nc.dram_tensor("wt", [128, KC * SH], fp8, kind="ExternalInput")
    ht = nc.dram_tensor("ht", [128, KC * NTOK], fp8, kind="ExternalInput")
    cidx = nc.dram_tensor("cidx", [128, NG * NSUB * KSLOT], i16, kind="ExternalInput")
    cval = nc.dram_tensor("cval", [128, NG * NSUB * KSLOT], bf16, kind="ExternalInput")
    w2v = nc.dram_tensor("w2v", [128, NG * NSUB * KSLOT], bf16, kind="ExternalInput")
    out = nc.dram_tensor("out", [128, NG * 3], f32, kind="ExternalOutput")

    with tile.TileContext(nc) as tc:
        with (
            tc.tile_pool(name="const", bufs=1) as cpool,
            tc.tile_pool(name="cw", bufs=2) as cwpool,
            tc.tile_pool(name="stage", bufs=2) as spool,
            tc.tile_pool(name="ps", bufs=4, space="PSUM") as ppool,
            tc.tile_pool(name="work", bufs=1) as wpool,
        ):
            wt_t = cpool.tile([128, KC, SH], fp8)
            nc.sync.dma_start(out=wt_t[:], in_=wt[:].rearrange("p (c v) -> p c v", c=KC))
            ht_t = cpool.tile([128, KC, NTOK], fp8)
            nc.sync.dma_start(out=ht_t[:], in_=ht[:].rearrange("p (c t) -> p c t", c=KC))
            cidx_t = cpool.tile([128, NG, NSUB, KSLOT], i16)
            nc.sync.dma_start(
                out=cidx_t[:],
                in_=cidx[:].rearrange("p (g s k) -> p g s k", g=NG, s=NSUB),
            )
            cval_t = cpool.tile([128, NG, NSUB, KSLOT], bf16)
            nc.sync.dma_start(
                out=cval_t[:],
                in_=cval[:].rearrange("p (g s k) -> p g s k", g=NG, s=NSUB),
            )
            w2v_t = cpool.tile([128, NG, NSUB, KSLOT], bf16)
            nc.sync.dma_start(
                out=w2v_t[:],
                in_=w2v[:].rearrange("p (g s k) -> p g s k", g=NG, s=NSUB),
            )

            outt = wpool.tile([128, NG * 3], f32)

            for g in range(NG):
                C = cwpool.tile([128, SH], bf16, tag="C")
                W2 = cwpool.tile([128, SH], bf16, tag="W2")
                for s in range(NSUB):
                    nc.gpsimd.local_scatter(
                        out_ap=C[:, s * SUBW : (s + 1) * SUBW],
                        data_ap=cval_t[:, g, s, :],
                        idxs_ap=cidx_t[:, g, s, :],
                        channels=128,
                        num_elems=SUBW,
                        num_idxs=KSLOT,
                    )
                    nc.gpsimd.local_scatter(
                        out_ap=W2[:, s * SUBW : (s + 1) * SUBW],
                        data_ap=w2v_t[:, g, s, :],
                        idxs_ap=cidx_t[:, g, s, :],
                        channels=128,
                        num_elems=SUBW,
                        num_idxs=KSLOT,
                    )

                stage = spool.tile([128, SH], f32, tag="st")
                for nt in range(NT):
                    w = min(512, SH - nt * 512)
                    ps = ppool.tile([128, 512], f32, tag="ps")
                    for c in range(KC):
                        nc.tensor.matmul(
                            out=ps[:, :w],
                            lhsT=ht_t[:, c, g * TPG : (g + 1) * TPG],
                            rhs=wt_t[:, c, nt * 512 : nt * 512 + w],
                            start=(c == 0),
                            stop=(c == KC - 1),
                        )
                    nc.scalar.copy(out=stage[:, nt * 512 : nt * 512 + w], in_=ps[:, :w])

                negmax = outt[:, 3 * g : 3 * g + 1]
                nc.vector.tensor_reduce(
                    out=negmax, in_=stage[:], axis=AX, op=OP.max, negate=True
                )
                # L2 = sum(stage * W2)  (product parked in-place in W2, bf16)
                nc.vector.tensor_tensor(out=W2[:], in0=stage[:], in1=W2[:], op=OP.mult)
                nc.vector.tensor_reduce(
                    out=outt[:, 3 * g + 2 : 3 * g + 3], in_=W2[:], axis=AX, op=OP.add
                )
                # stage <- exp(stage - max)
                nc.scalar.activation(
                    out=stage[:], in_=stage[:], func=ACTF.Exp, bias=negmax
                )
                # Z = sum(C * exp)  (product parked in-place in C, bf16)
                nc.vector.tensor_tensor(out=C[:], in0=stage[:], in1=C[:], op=OP.mult)
                nc.vector.tensor_reduce(
                    out=outt[:, 3 * g + 1 : 3 * g + 2], in_=C[:], axis=AX, op=OP.add
                )

            nc.sync.dma_start(out=out[:], in_=outt[:])

    nc.compile()
    return nc


def _prep_inputs(hidden_states, weight, target, noise_indx):
    h = np.asarray(hidden_states, np.float32).reshape(NTOK, H)
    W = np.asarray(weight, np.float32)
    tgt = np.asarray(target).reshape(NTOK).astype(np.int64)
    nz = np.asarray(noise_indx).astype(np.int64)
    fp8 = ml_dtypes.float8_e4m3
    bf16 = ml_dtypes.bfloat16

    # ht: h^T split into 6 chunks of 128 h-dims -> [128, KC, NTOK], replicated
    htc = np.ascontiguousarray(
        h.T.reshape(KC, 128, NTOK).transpose(1, 0, 2)
    ).reshape(128, KC * NTOK).astype(fp8)

    # Packed sparse (local idx, count, w2) per (token, core, subrange)
    ids = np.concatenate([nz, tgt[:, None]], axis=1)  # [NTOK, 513]
    keys = (np.arange(NTOK, dtype=np.int64)[:, None] * (SH * NCORES) + ids).ravel()
    uk, cnt = np.unique(keys, return_counts=True)
    n_u = uk // (SH * NCORES)
    id_u = uk % (SH * NCORES)
    core_u = id_u // SH
    loc_u = id_u % SH
    sub_u = loc_u // SUBW
    lloc_u = loc_u % SUBW
    w2 = NPROB * cnt + (0.9 - NPROB) * (id_u == tgt[n_u])

    # slot index within each (token, core, subrange) run (uk is sorted)
    grp = (n_u * NCORES + core_u) * NSUB + sub_u
    starts = np.flatnonzero(np.diff(grp, prepend=-1))
    runid = np.cumsum(np.isin(np.arange(grp.shape[0]), starts)) - 1
    slot = np.arange(grp.shape[0]) - starts[runid]
    assert slot.max() < KSLOT, f"slot overflow: {slot.max()}"

    IDX = np.full((NTOK, NCORES, NSUB, KSLOT), -1, np.int16)
    CV = np.zeros((NTOK, NCORES, NSUB, KSLOT), np.float32)
    W2V = np.zeros((NTOK, NCORES, NSUB, KSLOT), np.float32)
    IDX[n_u, core_u, sub_u, slot] = lloc_u
    CV[n_u, core_u, sub_u, slot] = cnt
    W2V[n_u, core_u, sub_u, slot] = w2

    in_maps = []
    for c in range(NCORES):
        lo, hi = c * SH, min((c + 1) * SH, V)
        Wc = np.zeros((SH, H), np.float32)
        Wc[: hi - lo] = W[lo:hi]
        wtc = np.ascontiguousarray(
            Wc.T.reshape(KC, 128, SH).transpose(1, 0, 2)
        ).reshape(128, KC * SH).astype(fp8)
        # token t = g*128 + p  ->  partition p, group g
        cidx = np.ascontiguousarray(
            IDX[:, c].reshape(NG, TPG, NSUB, KSLOT).transpose(1, 0, 2, 3)
        ).reshape(128, NG * NSUB * KSLOT)
        cvalc = np.ascontiguousarray(
            CV[:, c].reshape(NG, TPG, NSUB, KSLOT).transpose(1, 0, 2, 3)
        ).reshape(128, NG * NSUB * KSLOT).astype(bf16)
        w2vc = np.ascontiguousarray(
            W2V[:, c].reshape(NG, TPG, NSUB, KSLOT).transpose(1, 0, 2, 3)
        ).reshape(128, NG * NSUB * KSLOT).astype(bf16)
        in_maps.append({"wt": wtc, "ht": htc, "cidx": cidx, "cval": cvalc, "w2v": w2vc})
    return in_maps


def _combine(results):
    # per core: [128, NG*3] -> token t = g*128 + p at [p, 3g:3g+3]
    nm = np.stack(
        [np.asarray(r["out"], np.float64).reshape(128, NG, 3) for r in results]
    )  # [NCORES, 128, NG, 3]
    m = -nm[..., 0]  # [NCORES, 128, NG]
    Z = nm[..., 1]
    L2 = nm[..., 2]
    M = m.max(axis=0)  # [128, NG]
    Ztot = (Z * np.exp(m - M[None])).sum(axis=0)
    loss = M + np.log(Ztot) - L2.sum(axis=0)  # [128, NG]
    return np.float32(loss.mean())


def kernel(hidden_states, weight, target, noise_indx):
    from concourse.bass_utils import run_bass_kernel_spmd

    if "nc" not in _CACHE:
        _CACHE["nc"] = _build_bass()
    nc = _CACHE["nc"]
    in_maps = _prep_inputs(hidden_states, weight, target, noise_indx)
    res = run_bass_kernel_spmd(nc, in_maps, core_ids=list(range(NCORES)))
    return _combine(res.results)
